# revision 46
# baseline (speedup 1.0000x reference)
"""Multi-head causal self-attention (B=1, S=4096, D=1024, H=16) on 8 TRN2
NeuronCores, tensor-parallel over heads (2 heads per core).

v2: all-bf16 dataflow engineered to the TimelineSim cost model.
  - qkv^T = (X @ W)^T via matmul(lhsT=W_tile, rhs=X^T tile); K needs no bias
    (constant-in-t score shifts cancel in softmax), V's bias is folded into
    bo on the host (bo' = bo + bv @ Wo), so only Q gets a bias add.
  - V is produced directly in [t, d] layout by a second matmul pass with
    X^T tiles as the stationary operand (lhsT=xt[:,t-tile], rhs=Wv k-tile),
    eliminating all on-device transposes.
  - scores^T [t, s] per head via matmul(lhsT=K^T tile, rhs=Q^T chunk), full
    diagonal trim (bf16 matmuls have no free-dim>=256 constraint).
  - softmax without max-subtraction; exp on ACT with 1/8 scale folded in;
    pt in bf16 so the diagonal mask multiply runs in DVE 2x mode.
  - P^T @ V via matmul(lhsT=vhat[t,d]+ones column, rhs=pt) -> numerator rows
    0-63 and denominator on row 64 of PSUM.
  - denominator reciprocal broadcast back via a K=1 ones matmul; divide on
    DVE; y^T partial = Wo^T @ out^T; bf16 partials DMAed out once per chunk;
    host sums the 8 partials, adds bo', transposes.
  - one fused ~1MB DMA per chunk each way; proj/qkv/V matmuls of neighboring
    chunks are interleaved into the attention group loop as PE filler so the
    PE never idles behind the ACT-bound exp cadence.
"""

import sys

sys.path.insert(0, "/opt/trn_rl_repo")

import functools
import numpy as np
import ml_dtypes

D = 1024
H = 16
HD = 64
NCORES = 8
HPC = H // NCORES  # heads per core = 2
P = 128
CH = 512  # s-chunk width
BF16 = ml_dtypes.bfloat16


def build_nc(S):
    import concourse.bacc as bacc
    import concourse.mybir as mybir
    from concourse import tile

    f32 = mybir.dt.float32
    f32r = mybir.dt.float32r
    bf16 = mybir.dt.bfloat16
    ADD = mybir.AluOpType.add
    EXP = mybir.ActivationFunctionType.Exp

    NCHUNK = S // CH
    NT = S // P  # number of 128-row t-tiles
    ND = D // P  # 8 d-tiles

    nc = bacc.Bacc("TRN2", target_bir_lowering=False, debug=False)

    xt_d = nc.dram_tensor("xt", [D, S], bf16, kind="ExternalInput")
    wqkv_d = nc.dram_tensor("wqkv", [D, 3 * HPC * HD], bf16, kind="ExternalInput")
    bq_d = nc.dram_tensor("bq", [HPC * HD], f32, kind="ExternalInput")
    wo_d = nc.dram_tensor("wo", [HPC * HD, D], bf16, kind="ExternalInput")
    masks_d = nc.dram_tensor("masks", [P, 4, CH], bf16, kind="ExternalInput")
    yt_d = nc.dram_tensor("yt", [D, S], bf16, kind="ExternalOutput")

    # chunk-granularity DRAM views: partition-major [p, dtile, s]
    xt_v = xt_d[:].rearrange("(dt p) s -> p dt s", p=P)
    yt_v = yt_d[:].rearrange("(dt p) s -> p dt s", p=P)
    wqkv_v = wqkv_d[:].rearrange("(dt p) c -> p dt c", p=P)

    with tile.TileContext(nc) as tc:
        with (
            tc.tile_pool(name="consts", bufs=1) as consts,
            tc.tile_pool(name="xtp", bufs=3) as xtp,
            tc.tile_pool(name="ptp", bufs=4) as ptp,
            tc.tile_pool(name="nmp", bufs=8) as nmp,
            tc.tile_pool(name="otp", bufs=4) as otp,
            tc.tile_pool(name="rcp", bufs=8) as rcp,
            tc.tile_pool(name="ytp", bufs=2) as ytp,
            tc.tile_pool(name="scp", bufs=2, space="PSUM") as scp,
            tc.tile_pool(name="avp", bufs=2, space="PSUM") as avp,
            tc.tile_pool(name="qyp", bufs=2, space="PSUM") as qyp,
        ):
            # ---- persistent SBUF ----
            wqkv_sb = consts.tile([P, ND, 3 * HPC * HD], bf16)
            bq_sb = consts.tile([P, 1], f32)
            wo_sb = consts.tile([HPC * HD, D], bf16)
            masks_sb = consts.tile([P, 4, CH], bf16)
            ones_sb = consts.tile([P, HD], f32r)
            qt_sb = consts.tile([P, S], bf16)  # Q^T: h0 parts 0-63, h1 64-127
            kt_sb = consts.tile([P, S], bf16)
            # V-hat per head: [t-part, NT tiles, 72] (cols 0-63 = V, 64 = ones)
            vhat = [
                consts.tile([P, NT, 72], bf16, tag=f"vhat{h}", name=f"vhat{h}")
                for h in range(HPC)
            ]

            nc.vector.memset(ones_sb[64:65, :], 1.0)
            for h in range(HPC):
                nc.vector.memset(vhat[h][:, :, 64:65], 1.0)

            xts = {}  # chunk j -> xt tile
            cur_host = [-1]  # chunk whose group loop is executing

            def copy_ps(j=None):
                """PSUM->SBUF copy engine: ACT while it has slack (early
                chunks are PE-bound), DVE once exp dominates ACT."""
                if cur_host[0] <= 0:
                    return nc.scalar.copy
                return nc.vector.tensor_copy

            def load_x(j, split=False):
                xt_t = xtp.tile([P, ND, CH], bf16, tag="xt", name="xt_t")
                if split:  # per-dtile loads interleaved with the weight
                    # halves: first matmul waits only xt-d0 + W-half-1
                    for d in range(ND):
                        nc.sync.dma_start(
                            xt_t[:, d, :], xt_v[:, d, j * CH : (j + 1) * CH]
                        )
                        if d == 0:
                            nc.sync.dma_start(
                                wqkv_sb[:, 0 : ND // 2, :],
                                wqkv_v[:, 0 : ND // 2, :],
                            )
                        if d == 2:
                            nc.sync.dma_start(
                                wqkv_sb[:, ND // 2 :, :], wqkv_v[:, ND // 2 :, :]
                            )
                        if d == 3:
                            nc.sync.dma_start(
                                bq_sb[:], bq_d[:].rearrange("(i p) -> p i", p=P)
                            )
                        if d == 5:
                            nc.sync.dma_start(masks_sb[:], masks_d[:])
                else:
                    nc.sync.dma_start(xt_t[:], xt_v[:, :, j * CH : (j + 1) * CH])
                xts[j] = xt_t

            def emit_qkv_c(j, c):
                """Q (c=0) or K (c=1) projection for s-chunk j: one psum tile."""
                xt_t = xts[j]
                ps = qyp.tile([P, CH], f32, tag="qy", name=f"qkps{c}")
                for d in range(ND):
                    nc.tensor.matmul(
                        ps[:],
                        wqkv_sb[:, d, c * P : (c + 1) * P],
                        xt_t[:, d, :],
                        start=(d == 0),
                        stop=(d == ND - 1),
                    )
                if c == 0:
                    if cur_host[0] <= 0:
                        nc.scalar.add(
                            qt_sb[:, j * CH : (j + 1) * CH], ps[:], bq_sb[:, 0:1]
                        )
                    else:
                        nc.vector.tensor_scalar(
                            out=qt_sb[:, j * CH : (j + 1) * CH],
                            in0=ps[:],
                            scalar1=bq_sb[:, 0:1],
                            scalar2=None,
                            op0=ADD,
                        )
                else:
                    copy_ps()(kt_sb[:, j * CH : (j + 1) * CH], ps[:])

            def emit_v(j, sub, vbox):
                """V[t, d] for 128-t subtile `sub` of chunk j, both heads.
                All four subtiles share one [P, 4, 128] psum tile (vbox)."""
                xt_t = xts[j]
                if not vbox:
                    vbox.append(qyp.tile([P, 4, P], f32, tag="qy", name="vps"))
                vps = vbox[0]
                for d in range(ND):
                    nc.tensor.matmul(
                        vps[:, sub, :],
                        xt_t[:, d, sub * P : (sub + 1) * P],
                        wqkv_sb[:, d, 2 * P : 3 * P],
                        start=(d == 0),
                        stop=(d == ND - 1),
                    )
                if sub == 3:
                    for h in range(HPC):
                        copy_ps()(
                            vhat[h][:, 4 * j : 4 * j + 4, 0:64],
                            vps[:, :, 64 * h : 64 * h + 64],
                        )

            def emit_proj(j, ot, e):
                """output projection for chunk j, d-tile e."""
                yt_ps = qyp.tile([P, CH], f32, tag="qy", name="ytps")
                nc.tensor.matmul(
                    yt_ps[:],
                    wo_sb[:, e * P : (e + 1) * P],
                    ot[:],
                    start=True,
                    stop=True,
                )
                yt_st = yt_stage[j % 2]
                copy_ps()(yt_st[:, e, :], yt_ps[:])
                if e == ND - 1:
                    nc.sync.dma_start(yt_v[:, :, j * CH : (j + 1) * CH], yt_st[:])

            yt_stage = [
                ytp.tile([P, ND, CH], bf16, tag="yt", name=f"ytst{i}")
                for i in range(2)
            ]

            # ---- prologue: weights in 2 half-DMAs interleaved with chunk-0
            # x per d-tile; K/Q/V matmuls interleaved per d-tile so PE
            # consumption stays behind the HWDGE-serialized DMA delivery ----
            load_x(0, split=True)
            kps = avp.tile([P, CH], f32, tag="av", name="kps")
            qps = avp.tile([P, CH], f32, tag="av", name="qps")
            vb0 = []
            vb0.append(qyp.tile([P, 4, P], f32, tag="qy", name="vps"))
            xt_t = xts[0]
            for d in range(ND):
                nc.tensor.matmul(
                    kps[:], wqkv_sb[:, d, P : 2 * P], xt_t[:, d, :],
                    start=(d == 0), stop=(d == ND - 1),
                )
                nc.tensor.matmul(
                    qps[:], wqkv_sb[:, d, 0:P], xt_t[:, d, :],
                    start=(d == 0), stop=(d == ND - 1),
                )
                for sub in range(4):
                    nc.tensor.matmul(
                        vb0[0][:, sub, :],
                        xt_t[:, d, sub * P : (sub + 1) * P],
                        wqkv_sb[:, d, 2 * P : 3 * P],
                        start=(d == 0), stop=(d == ND - 1),
                    )
            nc.scalar.copy(kt_sb[:, 0:CH], kps[:])
            nc.scalar.add(qt_sb[:, 0:CH], qps[:], bq_sb[:, 0:1])
            for h in range(HPC):
                nc.scalar.copy(
                    vhat[h][:, 0:4, 0:64], vb0[0][:, :, 64 * h : 64 * h + 64]
                )
            load_x(1, split=False)
            nc.sync.dma_start(wo_sb[:], wo_d[:])
            emit_qkv_c(1, 1)
            emit_qkv_c(1, 0)
            vb1 = []
            for sub in range(4):
                emit_v(1, sub, vb1)

            # ---- global filler-atom queues ----
            # Deferrable PE work sliced into ~200-450ns atoms, popped into the
            # attention group loop so the PE neither idles behind the
            # ACT-bound exp cadence (late chunks) nor bursts ahead of it
            # (early chunks, which are already PE-bound).
            #   mandq: qkv/V atoms, (deadline_chunk, cost, fn) — paced to
            #          finish during chunk deadline-1.
            #   defq:  div/proj atoms, (soft_deadline, cost, fn) — only
            #          emitted into ACT-bound chunks to cover the PE deficit.
            atomq = []  # mandatory FIFO
            defq = []  # deferrable FIFO

            def enqueue_qkv_atoms(jf):
                for c in (1, 0):
                    box = []

                    def mm(d, c=c, jf=jf, box=box):
                        if not box:
                            box.append(
                                qyp.tile([P, CH], f32, tag="qy", name=f"qkps{c}")
                            )
                        nc.tensor.matmul(
                            box[0][:],
                            wqkv_sb[:, d, c * P : (c + 1) * P],
                            xts[jf][:, d, :],
                            start=(d == 0),
                            stop=(d == ND - 1),
                        )

                    def cp(c=c, jf=jf, box=box):
                        if c == 0:
                            nc.vector.tensor_scalar(
                                out=qt_sb[:, jf * CH : (jf + 1) * CH],
                                in0=box[0][:],
                                scalar1=bq_sb[:, 0:1],
                                scalar2=None,
                                op0=ADD,
                            )
                        else:
                            nc.vector.tensor_copy(
                                kt_sb[:, jf * CH : (jf + 1) * CH], box[0][:]
                            )

                    for d in range(ND):
                        atomq.append((jf, 213, lambda d=d, mm=mm: mm(d)))
                    atomq.append((jf, 0, cp))
                vbox = []
                for sub in range(4):
                    atomq.append(
                        (jf, 427, lambda jf=jf, sub=sub, vbox=vbox: emit_v(jf, sub, vbox))
                    )

            def enqueue_divproj_atoms(jp, rcs, nms):
                ot_box = []

                def div_h(h):
                    if not ot_box:
                        ot_box.append(otp.tile([P, CH], bf16, tag="ot", name="ot"))
                    bc = qyp.tile([HD, CH], f32, tag="qy", name="bc")
                    nc.tensor.matmul(
                        bc[:],
                        ones_sb[64:65, 0:HD],
                        rcs[h][64:65, :],
                        start=True,
                        stop=True,
                    )
                    nc.vector.tensor_mul(
                        ot_box[0][64 * h : 64 * h + 64, :], nms[h][:], bc[:]
                    )

                dl = jp + 4  # soft deadline: keep SBUF rings bounded
                for h in range(HPC):
                    defq.append((dl, 213, lambda h=h: div_h(h)))
                for e in range(ND):
                    defq.append(
                        (dl, 213, lambda jp=jp, e=e: emit_proj(jp, ot_box[0], e))
                    )

            # division state carried across chunks: (rcs, nms) per head
            carried = None  # (j_prev, rcs, nms)

            for j in range(NCHUNK):
                cur_host[0] = j
                ntt = 4 * (j + 1)

                # anything due before this chunk runs: emit now (safety drain)
                while atomq and atomq[0][0] <= j:
                    atomq.pop(0)[2]()
                while defq and defq[0][0] <= j:
                    defq.pop(0)[2]()

                av = [
                    avp.tile([P, CH], f32, tag="av", name=f"av{h}")
                    for h in range(HPC)
                ]

                # next-next chunk's activations: DMA in flight ASAP
                if j + 2 < NCHUNK:
                    load_x(j + 2, split=False)
                if carried is not None:
                    enqueue_divproj_atoms(*carried)
                    carried = None
                if j + 2 < NCHUNK:
                    enqueue_qkv_atoms(j + 2)

                # mandatory: atoms due before chunk j+1, paced over this chunk
                mand = sum(1 for a in atomq if a[0] <= j + 1)
                # deferrable: only into ACT-bound chunks, sized to the per-
                # group PE deficit (ACT group cadence minus scores+AV time)
                defc = 190.0 if j >= 3 else 0.0

                def soff(tt):
                    o = (tt - 4 * j) * P if tt >= 4 * j else 0
                    return min(max(0, o), 3 * P)

                def flush(tt, sc):
                    """exp + mask + AV for t-tile tt (both heads)."""
                    o = soff(tt)
                    pt = ptp.tile([P, HPC, CH], bf16, tag="pt", name="pt")
                    sc_v = sc[:].rearrange("p (g c) -> p g c", c=CH)
                    nc.scalar.activation(
                        pt[:, :, o:], sc_v[:, :, o:], EXP, scale=0.125
                    )
                    if tt >= 4 * j:  # diagonal: one masked mul for both heads
                        k = tt - 4 * j
                        nc.vector.tensor_mul(
                            pt[:, :, o:],
                            pt[:, :, o:],
                            masks_sb[:, k : k + 1, o:].broadcast_to(
                                [P, HPC, CH - o]
                            ),
                        )
                    for h in range(HPC):
                        nc.tensor.matmul(
                            av[h][0:65, o:],
                            vhat[h][:, tt, 0:65],
                            pt[:, h, o:],
                            start=(tt == 0),
                            stop=(tt == ntt - 1),
                        )

                pending = None
                for tt in range(ntt):
                    o = soff(tt)
                    sc = scp.tile([P, HPC * CH], f32, tag="sc", name="sc")
                    for h in range(HPC):
                        nc.tensor.matmul(
                            sc[:, h * CH + o : (h + 1) * CH],
                            kt_sb[64 * h : 64 * h + 64, tt * P : (tt + 1) * P],
                            qt_sb[64 * h : 64 * h + 64, j * CH + o : (j + 1) * CH],
                            start=True,
                            stop=True,
                        )
                    if pending is not None:
                        flush(*pending)
                    pending = (tt, sc)
                    # pop filler atoms: deadline-paced + deficit-leveled
                    iters_left = ntt - tt
                    need = -(-mand // iters_left) if mand > 0 else 0
                    spent = 0.0
                    while atomq and need > 0:
                        dl, cost, fn = atomq.pop(0)
                        fn()
                        spent += cost
                        mand -= 1
                        need -= 1
                    while defq and spent < defc:
                        dl, cost, fn = defq.pop(0)
                        fn()
                        spent += cost
                if pending is not None:
                    flush(*pending)

                if j == NCHUNK - 1:  # drain all remaining atoms before tail
                    while atomq:
                        atomq.pop(0)[2]()
                    while defq:
                        defq.pop(0)[2]()

                if j < NCHUNK - 1:
                    # ---- reciprocals + numerator copies (free av tiles) ----
                    rcs, nms = [], []
                    for h in range(HPC):
                        rc = rcp.tile([P, CH], f32r, tag="rc", name="rc")
                        with nc.allow_low_precision(
                            "fp32r recip feeds fp22 matmul"
                        ):
                            nc.vector.reciprocal(rc[64:65, :], av[h][64:65, :])
                        nm = nmp.tile([HD, CH], f32, tag="nm", name="nm")
                        nc.vector.tensor_copy(nm[:], av[h][0:64, :])
                        rcs.append(rc)
                        nms.append(nm)
                    carried = (j, rcs, nms)
                else:
                    av_last = av

            # ---- epilogue: last chunk's division + projection, pipelined in
            # two independent half-width (256-col) chains so DVE/ACT/PE/DMA
            # overlap down the tail ----
            jp = NCHUNK - 1
            HC = CH // 2
            ot = otp.tile([P, CH], bf16, tag="ot", name="ot")
            yt_st = yt_stage[jp % 2]
            for half in range(2):
                sl = slice(half * HC, half * HC + HC)
                rcs = []
                for h in range(HPC):
                    rc = rcp.tile([P, CH], f32r, tag="rc", name="rc")
                    with nc.allow_low_precision("fp32r recip feeds fp22 matmul"):
                        nc.vector.reciprocal(rc[64:65, sl], av_last[h][64:65, sl])
                    nm = nmp.tile([HD, CH], f32, tag="nm", name="nm")
                    nc.scalar.copy(nm[:, sl], av_last[h][0:64, sl])
                    bc = qyp.tile([HD, CH], f32, tag="qy", name="bc")
                    nc.tensor.matmul(
                        bc[:, sl],
                        ones_sb[64:65, 0:HD],
                        rc[64:65, sl],
                        start=True,
                        stop=True,
                    )
                    nc.vector.tensor_mul(
                        ot[64 * h : 64 * h + 64, sl], nm[:, sl], bc[:, sl]
                    )
                for e in range(ND):
                    if e % 4 == 0:
                        tail_ps = scp.tile([P, HPC * CH], f32, tag="sc", name="sc")
                    yt_ps = tail_ps[:, (e % 4) * HC : (e % 4 + 1) * HC]
                    nc.tensor.matmul(
                        yt_ps,
                        wo_sb[:, e * P : (e + 1) * P],
                        ot[:, sl],
                        start=True,
                        stop=True,
                    )
                    if e % 2 == 1:
                        nc.scalar.copy(yt_st[:, e, sl], yt_ps)
                    else:
                        nc.vector.tensor_copy(yt_st[:, e, sl], yt_ps)
                    if e % 4 == 3:
                        lo = jp * CH + half * HC
                        nc.sync.dma_start(
                            yt_v[:, e - 3 : e + 1, lo : lo + HC],
                            yt_st[:, e - 3 : e + 1, sl],
                        )

    return nc


@functools.lru_cache(maxsize=2)
def _get_nc(S):
    nc = build_nc(S)
    nc.compile()
    return nc


def make_in_maps(input, Wqkv, bqkv, Wo, S):
    """Host-side shard prep. input [1,S,D] (or [S,D]); returns per-core dicts."""
    x = np.asarray(input, dtype=np.float32).reshape(S, D)
    xt = np.ascontiguousarray(x.T.astype(BF16))
    Wqkv = np.asarray(Wqkv, dtype=np.float32)
    bqkv = np.asarray(bqkv, dtype=np.float32)
    Wo = np.asarray(Wo, dtype=np.float32)

    # causal masks for the 4 diagonal 128-blocks of a 512 chunk
    pp = np.arange(P)[:, None]
    ff = np.arange(CH)[None, :]
    masks = np.stack(
        [(ff >= pp + P * k).astype(BF16) for k in range(4)], axis=1
    )  # [128, 4, 512]
    masks = np.ascontiguousarray(masks)

    Wq, Wk, Wv = Wqkv[:, 0:D], Wqkv[:, D : 2 * D], Wqkv[:, 2 * D : 3 * D]
    bq = bqkv[0:D]

    in_maps = []
    for c in range(NCORES):
        hs = [c * HPC + i for i in range(HPC)]
        cols = lambda W: np.concatenate(
            [W[:, h * HD : (h + 1) * HD] for h in hs], axis=1
        )
        colsb = lambda b: np.concatenate(
            [b[h * HD : (h + 1) * HD] for h in hs], axis=0
        )
        wqkv_l = np.ascontiguousarray(
            np.concatenate([cols(Wq), cols(Wk), cols(Wv)], axis=1).astype(BF16)
        )
        bq_l = np.ascontiguousarray(colsb(bq).astype(np.float32))
        wo_l = np.ascontiguousarray(
            Wo[hs[0] * HD : hs[0] * HD + HPC * HD, :].astype(BF16)
        )
        in_maps.append(
            {
                "xt": xt,
                "wqkv": wqkv_l,
                "bq": bq_l,
                "wo": wo_l,
                "masks": masks,
            }
        )
    return in_maps


def kernel(input, Wqkv, bqkv, Wo, bo):
    from concourse.bass_utils import run_bass_kernel_spmd

    S = np.asarray(input).reshape(-1, D).shape[0]
    nc = _get_nc(S)
    in_maps = make_in_maps(input, Wqkv, bqkv, Wo, S)
    res = None
    last_exc = None
    for _attempt in range(3):  # transient NRT/device errors: retry
        try:
            res = run_bass_kernel_spmd(nc, in_maps, core_ids=list(range(NCORES)))
            break
        except Exception as e:  # noqa: BLE001
            last_exc = e
    if res is None:
        raise last_exc
    yt = res.results[0]["yt"].astype(np.float32)
    for r in res.results[1:]:
        yt += r["yt"].astype(np.float32)
    # fold the V bias through the output projection: y += bv @ Wo + bo
    bv = np.asarray(bqkv, dtype=np.float32)[2 * D : 3 * D]
    bo_eff = np.asarray(bo, dtype=np.float32) + bv @ np.asarray(
        Wo, dtype=np.float32
    )
    y = yt.T + bo_eff[None, :]
    return np.ascontiguousarray(y, dtype=np.float32).reshape(1, S, D)


# revision 47
# speedup vs baseline: 1.0112x; 1.0112x over previous
"""Multi-head causal self-attention (B=1, S=4096, D=1024, H=16) on 8 TRN2
NeuronCores, tensor-parallel over heads (2 heads per core).

v2: all-bf16 dataflow engineered to the TimelineSim cost model.
  - qkv^T = (X @ W)^T via matmul(lhsT=W_tile, rhs=X^T tile); K needs no bias
    (constant-in-t score shifts cancel in softmax), V's bias is folded into
    bo on the host (bo' = bo + bv @ Wo), so only Q gets a bias add.
  - V is produced directly in [t, d] layout by a second matmul pass with
    X^T tiles as the stationary operand (lhsT=xt[:,t-tile], rhs=Wv k-tile),
    eliminating all on-device transposes.
  - scores^T [t, s] per head via matmul(lhsT=K^T tile, rhs=Q^T chunk), full
    diagonal trim (bf16 matmuls have no free-dim>=256 constraint).
  - softmax without max-subtraction; exp on ACT with 1/8 scale folded in;
    pt in bf16 so the diagonal mask multiply runs in DVE 2x mode.
  - P^T @ V via matmul(lhsT=vhat[t,d]+ones column, rhs=pt) -> numerator rows
    0-63 and denominator on row 64 of PSUM.
  - denominator reciprocal broadcast back via a K=1 ones matmul; divide on
    DVE; y^T partial = Wo^T @ out^T; bf16 partials DMAed out once per chunk;
    host sums the 8 partials, adds bo', transposes.
  - one fused ~1MB DMA per chunk each way; proj/qkv/V matmuls of neighboring
    chunks are interleaved into the attention group loop as PE filler so the
    PE never idles behind the ACT-bound exp cadence.
"""

import sys

sys.path.insert(0, "/opt/trn_rl_repo")

import functools
import numpy as np
import ml_dtypes

D = 1024
H = 16
HD = 64
NCORES = 8
HPC = H // NCORES  # heads per core = 2
P = 128
CH = 512  # s-chunk width
BF16 = ml_dtypes.bfloat16


def build_nc(S):
    import concourse.bacc as bacc
    import concourse.mybir as mybir
    from concourse import tile

    f32 = mybir.dt.float32
    f32r = mybir.dt.float32r
    bf16 = mybir.dt.bfloat16
    ADD = mybir.AluOpType.add
    EXP = mybir.ActivationFunctionType.Exp

    NCHUNK = S // CH
    NT = S // P  # number of 128-row t-tiles
    ND = D // P  # 8 d-tiles

    nc = bacc.Bacc("TRN2", target_bir_lowering=False, debug=False)

    xt_d = nc.dram_tensor("xt", [D, S], bf16, kind="ExternalInput")
    wqkv_d = nc.dram_tensor("wqkv", [D, 3 * HPC * HD], bf16, kind="ExternalInput")
    bq_d = nc.dram_tensor("bq", [HPC * HD], f32, kind="ExternalInput")
    wo_d = nc.dram_tensor("wo", [HPC * HD, D], bf16, kind="ExternalInput")
    masks_d = nc.dram_tensor("masks", [P, 4, CH], bf16, kind="ExternalInput")
    yt_d = nc.dram_tensor("yt", [D, S], bf16, kind="ExternalOutput")

    # chunk-granularity DRAM views: partition-major [p, dtile, s]
    xt_v = xt_d[:].rearrange("(dt p) s -> p dt s", p=P)
    yt_v = yt_d[:].rearrange("(dt p) s -> p dt s", p=P)
    wqkv_v = wqkv_d[:].rearrange("(dt p) c -> p dt c", p=P)

    with tile.TileContext(nc) as tc:
        with (
            tc.tile_pool(name="consts", bufs=1) as consts,
            tc.tile_pool(name="xtp", bufs=3) as xtp,
            tc.tile_pool(name="ptp", bufs=4) as ptp,
            tc.tile_pool(name="nmp", bufs=8) as nmp,
            tc.tile_pool(name="otp", bufs=4) as otp,
            tc.tile_pool(name="rcp", bufs=8) as rcp,
            tc.tile_pool(name="ytp", bufs=2) as ytp,
            tc.tile_pool(name="scp", bufs=2, space="PSUM") as scp,
            tc.tile_pool(name="avp", bufs=2, space="PSUM") as avp,
            tc.tile_pool(name="qyp", bufs=2, space="PSUM") as qyp,
        ):
            # ---- persistent SBUF ----
            wqkv_sb = consts.tile([P, ND, 3 * HPC * HD], bf16)
            bq_sb = consts.tile([P, 1], f32)
            wo_sb = consts.tile([HPC * HD, D], bf16)
            masks_sb = consts.tile([P, 4, CH], bf16)
            ones_sb = consts.tile([P, HD], f32r)
            qt_sb = consts.tile([P, S], bf16)  # Q^T: h0 parts 0-63, h1 64-127
            kt_sb = consts.tile([P, S], bf16)
            # V-hat per head: [t-part, NT tiles, 72] (cols 0-63 = V, 64 = ones)
            vhat = [
                consts.tile([P, NT, 72], bf16, tag=f"vhat{h}", name=f"vhat{h}")
                for h in range(HPC)
            ]

            nc.vector.memset(ones_sb[64:65, :], 1.0)
            for h in range(HPC):
                nc.vector.memset(vhat[h][:, :, 64:65], 1.0)

            xts = {}  # chunk j -> xt tile
            cur_host = [-1]  # chunk whose group loop is executing

            def copy_ps(j=None):
                """PSUM->SBUF copy engine: ACT while it has slack (early
                chunks are PE-bound), DVE once exp dominates ACT."""
                if cur_host[0] <= 0:
                    return nc.scalar.copy
                return nc.vector.tensor_copy

            def load_x(j, split=False):
                xt_t = xtp.tile([P, ND, CH], bf16, tag="xt", name="xt_t")
                if split:  # per-dtile loads interleaved with the weight
                    # halves: first matmul waits only xt-d0 + W-half-1
                    for d in range(ND):
                        nc.sync.dma_start(
                            xt_t[:, d, :], xt_v[:, d, j * CH : (j + 1) * CH]
                        )
                        if d == 0:
                            nc.sync.dma_start(
                                wqkv_sb[:, 0 : ND // 2, :],
                                wqkv_v[:, 0 : ND // 2, :],
                            )
                        if d == 2:
                            nc.sync.dma_start(
                                wqkv_sb[:, ND // 2 :, :], wqkv_v[:, ND // 2 :, :]
                            )
                        if d == 3:
                            nc.sync.dma_start(
                                bq_sb[:], bq_d[:].rearrange("(i p) -> p i", p=P)
                            )
                        if d == 5:
                            nc.sync.dma_start(masks_sb[:], masks_d[:])
                else:
                    nc.sync.dma_start(xt_t[:], xt_v[:, :, j * CH : (j + 1) * CH])
                xts[j] = xt_t

            def emit_qkv_c(j, c):
                """Q (c=0) or K (c=1) projection for s-chunk j: one psum tile."""
                xt_t = xts[j]
                ps = qyp.tile([P, CH], f32, tag="qy", name=f"qkps{c}")
                for d in range(ND):
                    nc.tensor.matmul(
                        ps[:],
                        wqkv_sb[:, d, c * P : (c + 1) * P],
                        xt_t[:, d, :],
                        start=(d == 0),
                        stop=(d == ND - 1),
                    )
                if c == 0:
                    if cur_host[0] <= 0:
                        nc.scalar.add(
                            qt_sb[:, j * CH : (j + 1) * CH], ps[:], bq_sb[:, 0:1]
                        )
                    else:
                        nc.vector.tensor_scalar(
                            out=qt_sb[:, j * CH : (j + 1) * CH],
                            in0=ps[:],
                            scalar1=bq_sb[:, 0:1],
                            scalar2=None,
                            op0=ADD,
                        )
                else:
                    copy_ps()(kt_sb[:, j * CH : (j + 1) * CH], ps[:])

            def emit_v(j, sub, vbox):
                """V[t, d] for 128-t subtile `sub` of chunk j, both heads.
                All four subtiles share one [P, 4, 128] psum tile (vbox)."""
                xt_t = xts[j]
                if not vbox:
                    vbox.append(qyp.tile([P, 4, P], f32, tag="qy", name="vps"))
                vps = vbox[0]
                for d in range(ND):
                    nc.tensor.matmul(
                        vps[:, sub, :],
                        xt_t[:, d, sub * P : (sub + 1) * P],
                        wqkv_sb[:, d, 2 * P : 3 * P],
                        start=(d == 0),
                        stop=(d == ND - 1),
                    )
                if sub == 3:
                    for h in range(HPC):
                        copy_ps()(
                            vhat[h][:, 4 * j : 4 * j + 4, 0:64],
                            vps[:, :, 64 * h : 64 * h + 64],
                        )

            def emit_proj(j, ot, e):
                """output projection for chunk j, d-tile e."""
                yt_ps = qyp.tile([P, CH], f32, tag="qy", name="ytps")
                nc.tensor.matmul(
                    yt_ps[:],
                    wo_sb[:, e * P : (e + 1) * P],
                    ot[:],
                    start=True,
                    stop=True,
                )
                yt_st = yt_stage[j % 2]
                copy_ps()(yt_st[:, e, :], yt_ps[:])
                if e == ND - 1:
                    nc.sync.dma_start(yt_v[:, :, j * CH : (j + 1) * CH], yt_st[:])

            yt_stage = [
                ytp.tile([P, ND, CH], bf16, tag="yt", name=f"ytst{i}")
                for i in range(2)
            ]

            # ---- prologue: weights in 2 half-DMAs interleaved with chunk-0
            # x per d-tile; K/Q/V matmuls interleaved per d-tile so PE
            # consumption stays behind the HWDGE-serialized DMA delivery ----
            load_x(0, split=True)
            kps = avp.tile([P, CH], f32, tag="av", name="kps")
            qps = avp.tile([P, CH], f32, tag="av", name="qps")
            vb0 = []
            vb0.append(qyp.tile([P, 4, P], f32, tag="qy", name="vps"))
            xt_t = xts[0]
            for d in range(ND):
                nc.tensor.matmul(
                    kps[:], wqkv_sb[:, d, P : 2 * P], xt_t[:, d, :],
                    start=(d == 0), stop=(d == ND - 1),
                )
                nc.tensor.matmul(
                    qps[:], wqkv_sb[:, d, 0:P], xt_t[:, d, :],
                    start=(d == 0), stop=(d == ND - 1),
                )
                for sub in range(4):
                    nc.tensor.matmul(
                        vb0[0][:, sub, :],
                        xt_t[:, d, sub * P : (sub + 1) * P],
                        wqkv_sb[:, d, 2 * P : 3 * P],
                        start=(d == 0), stop=(d == ND - 1),
                    )
            nc.scalar.copy(kt_sb[:, 0:CH], kps[:])
            nc.scalar.add(qt_sb[:, 0:CH], qps[:], bq_sb[:, 0:1])
            for h in range(HPC):
                nc.scalar.copy(
                    vhat[h][:, 0:4, 0:64], vb0[0][:, :, 64 * h : 64 * h + 64]
                )
            load_x(1, split=False)
            nc.sync.dma_start(wo_sb[:], wo_d[:])
            emit_qkv_c(1, 1)
            emit_qkv_c(1, 0)
            vb1 = []
            for sub in range(4):
                emit_v(1, sub, vb1)

            # ---- global filler-atom queues ----
            # Deferrable PE work sliced into ~200-450ns atoms, popped into the
            # attention group loop so the PE neither idles behind the
            # ACT-bound exp cadence (late chunks) nor bursts ahead of it
            # (early chunks, which are already PE-bound).
            #   mandq: qkv/V atoms, (deadline_chunk, cost, fn) — paced to
            #          finish during chunk deadline-1.
            #   defq:  div/proj atoms, (soft_deadline, cost, fn) — only
            #          emitted into ACT-bound chunks to cover the PE deficit.
            atomq = []  # mandatory FIFO
            defq = []  # deferrable FIFO

            def enqueue_qkv_atoms(jf):
                for c in (1, 0):
                    box = []

                    def mm(d, c=c, jf=jf, box=box):
                        if not box:
                            box.append(
                                qyp.tile([P, CH], f32, tag="qy", name=f"qkps{c}")
                            )
                        nc.tensor.matmul(
                            box[0][:],
                            wqkv_sb[:, d, c * P : (c + 1) * P],
                            xts[jf][:, d, :],
                            start=(d == 0),
                            stop=(d == ND - 1),
                        )

                    def cp(c=c, jf=jf, box=box):
                        if c == 0:
                            nc.vector.tensor_scalar(
                                out=qt_sb[:, jf * CH : (jf + 1) * CH],
                                in0=box[0][:],
                                scalar1=bq_sb[:, 0:1],
                                scalar2=None,
                                op0=ADD,
                            )
                        else:
                            nc.vector.tensor_copy(
                                kt_sb[:, jf * CH : (jf + 1) * CH], box[0][:]
                            )

                    for d in range(ND):
                        atomq.append((jf, 213, lambda d=d, mm=mm: mm(d)))
                    atomq.append((jf, 0, cp))
                vbox = []
                for sub in range(4):
                    atomq.append(
                        (jf, 427, lambda jf=jf, sub=sub, vbox=vbox: emit_v(jf, sub, vbox))
                    )

            def enqueue_divproj_atoms(jp, rcs, nms):
                ot_box = []

                def div_h(h):
                    if not ot_box:
                        ot_box.append(otp.tile([P, CH], bf16, tag="ot", name="ot"))
                    bc = qyp.tile([HD, CH], f32, tag="qy", name="bc")
                    nc.tensor.matmul(
                        bc[:],
                        ones_sb[64:65, 0:HD],
                        rcs[h][64:65, :],
                        start=True,
                        stop=True,
                    )
                    nc.vector.tensor_mul(
                        ot_box[0][64 * h : 64 * h + 64, :], nms[h][:], bc[:]
                    )

                dl = jp + 4  # soft deadline: keep SBUF rings bounded
                for h in range(HPC):
                    defq.append((dl, 213, lambda h=h: div_h(h)))
                for e in range(ND):
                    defq.append(
                        (dl, 213, lambda jp=jp, e=e: emit_proj(jp, ot_box[0], e))
                    )

            # division state carried across chunks: (rcs, nms) per head
            carried = None  # (j_prev, rcs, nms)

            for j in range(NCHUNK):
                cur_host[0] = j
                ntt = 4 * (j + 1)

                # anything due before this chunk runs: emit now (safety drain)
                while atomq and atomq[0][0] <= j:
                    atomq.pop(0)[2]()
                while defq and defq[0][0] <= j:
                    defq.pop(0)[2]()

                av = [
                    avp.tile([P, CH], f32, tag="av", name=f"av{h}")
                    for h in range(HPC)
                ]

                # next-next chunk's activations: DMA in flight ASAP
                if j + 2 < NCHUNK:
                    load_x(j + 2, split=False)
                if carried is not None:
                    enqueue_divproj_atoms(*carried)
                    carried = None
                if j + 2 < NCHUNK:
                    enqueue_qkv_atoms(j + 2)

                # mandatory: atoms due before chunk j+1, paced over this chunk
                mand = sum(1 for a in atomq if a[0] <= j + 1)
                # deferrable: only into ACT-bound chunks, sized to the per-
                # group PE deficit (ACT group cadence minus scores+AV time)
                defc = 190.0 if j >= 3 else 0.0

                def soff(tt):
                    o = (tt - 4 * j) * P if tt >= 4 * j else 0
                    return min(max(0, o), 3 * P)

                def flush(tt, sc):
                    """exp + mask + AV for t-tile tt (both heads)."""
                    o = soff(tt)
                    pt = ptp.tile([P, HPC, CH], bf16, tag="pt", name="pt")
                    sc_v = sc[:].rearrange("p (g c) -> p g c", c=CH)
                    nc.scalar.activation(
                        pt[:, :, o:], sc_v[:, :, o:], EXP, scale=0.125
                    )
                    if tt >= 4 * j:  # diagonal: one masked mul for both heads
                        k = tt - 4 * j
                        nc.vector.tensor_mul(
                            pt[:, :, o:],
                            pt[:, :, o:],
                            masks_sb[:, k : k + 1, o:].broadcast_to(
                                [P, HPC, CH - o]
                            ),
                        )
                    for h in range(HPC):
                        nc.tensor.matmul(
                            av[h][0:65, o:],
                            vhat[h][:, tt, 0:65],
                            pt[:, h, o:],
                            start=(tt == 0),
                            stop=(tt == ntt - 1),
                        )

                pending = None
                for tt in range(ntt):
                    o = soff(tt)
                    sc = scp.tile([P, HPC * CH], f32, tag="sc", name="sc")
                    for h in range(HPC):
                        nc.tensor.matmul(
                            sc[:, h * CH + o : (h + 1) * CH],
                            kt_sb[64 * h : 64 * h + 64, tt * P : (tt + 1) * P],
                            qt_sb[64 * h : 64 * h + 64, j * CH + o : (j + 1) * CH],
                            start=True,
                            stop=True,
                        )
                    if pending is not None:
                        flush(*pending)
                    pending = (tt, sc)
                    # pop filler atoms: deadline-paced + deficit-leveled
                    iters_left = ntt - tt
                    need = -(-mand // iters_left) if mand > 0 else 0
                    spent = 0.0
                    while atomq and need > 0:
                        dl, cost, fn = atomq.pop(0)
                        fn()
                        spent += cost
                        mand -= 1
                        need -= 1
                    while defq and spent < defc:
                        dl, cost, fn = defq.pop(0)
                        fn()
                        spent += cost
                if pending is not None:
                    flush(*pending)

                if j == NCHUNK - 1:  # drain all remaining atoms before tail
                    while atomq:
                        atomq.pop(0)[2]()
                    while defq:
                        defq.pop(0)[2]()

                if j < NCHUNK - 1:
                    # ---- reciprocals + numerator copies (free av tiles) ----
                    rcs, nms = [], []
                    for h in range(HPC):
                        rc = rcp.tile([P, CH], f32r, tag="rc", name="rc")
                        with nc.allow_low_precision(
                            "fp32r recip feeds fp22 matmul"
                        ):
                            nc.vector.reciprocal(rc[64:65, :], av[h][64:65, :])
                        nm = nmp.tile([HD, CH], f32, tag="nm", name="nm")
                        nc.vector.tensor_copy(nm[:], av[h][0:64, :])
                        rcs.append(rc)
                        nms.append(nm)
                    carried = (j, rcs, nms)
                else:
                    av_last = av

            # ---- epilogue: last chunk's division + projection, pipelined in
            # two independent half-width (256-col) chains so DVE/ACT/PE/DMA
            # overlap down the tail ----
            jp = NCHUNK - 1
            HC = CH // 2
            ot = otp.tile([P, CH], bf16, tag="ot", name="ot")
            yt_st = yt_stage[jp % 2]
            for half in range(2):
                sl = slice(half * HC, half * HC + HC)
                rcs = []
                for h in range(HPC):
                    rc = rcp.tile([P, CH], f32r, tag="rc", name="rc")
                    with nc.allow_low_precision("fp32r recip feeds fp22 matmul"):
                        nc.vector.reciprocal(rc[64:65, sl], av_last[h][64:65, sl])
                    nm = nmp.tile([HD, CH], f32, tag="nm", name="nm")
                    nc.scalar.copy(nm[:, sl], av_last[h][0:64, sl])
                    bc = qyp.tile([HD, CH], f32, tag="qy", name="bc")
                    nc.tensor.matmul(
                        bc[:, sl],
                        ones_sb[64:65, 0:HD],
                        rc[64:65, sl],
                        start=True,
                        stop=True,
                    )
                    nc.vector.tensor_mul(
                        ot[64 * h : 64 * h + 64, sl], nm[:, sl], bc[:, sl]
                    )
                for e in range(ND):
                    if e % 2 == 0:
                        yt_ps = qyp.tile([P, CH], f32, tag="qy", name="ytps")[
                            :, 0:HC
                        ]
                    else:
                        yt_ps = scp.tile([P, HPC * CH], f32, tag="sc", name="sc")[
                            :, 0:HC
                        ]
                    nc.tensor.matmul(
                        yt_ps,
                        wo_sb[:, e * P : (e + 1) * P],
                        ot[:, sl],
                        start=True,
                        stop=True,
                    )
                    if e % 2 == 1:
                        nc.scalar.copy(yt_st[:, e, sl], yt_ps)
                    else:
                        nc.vector.tensor_copy(yt_st[:, e, sl], yt_ps)
                    if e % 4 == 3:
                        lo = jp * CH + half * HC
                        nc.sync.dma_start(
                            yt_v[:, e - 3 : e + 1, lo : lo + HC],
                            yt_st[:, e - 3 : e + 1, sl],
                        )

    return nc


@functools.lru_cache(maxsize=2)
def _get_nc(S):
    nc = build_nc(S)
    nc.compile()
    return nc


def make_in_maps(input, Wqkv, bqkv, Wo, S):
    """Host-side shard prep. input [1,S,D] (or [S,D]); returns per-core dicts."""
    x = np.asarray(input, dtype=np.float32).reshape(S, D)
    xt = np.ascontiguousarray(x.T.astype(BF16))
    Wqkv = np.asarray(Wqkv, dtype=np.float32)
    bqkv = np.asarray(bqkv, dtype=np.float32)
    Wo = np.asarray(Wo, dtype=np.float32)

    # causal masks for the 4 diagonal 128-blocks of a 512 chunk
    pp = np.arange(P)[:, None]
    ff = np.arange(CH)[None, :]
    masks = np.stack(
        [(ff >= pp + P * k).astype(BF16) for k in range(4)], axis=1
    )  # [128, 4, 512]
    masks = np.ascontiguousarray(masks)

    Wq, Wk, Wv = Wqkv[:, 0:D], Wqkv[:, D : 2 * D], Wqkv[:, 2 * D : 3 * D]
    bq = bqkv[0:D]

    in_maps = []
    for c in range(NCORES):
        hs = [c * HPC + i for i in range(HPC)]
        cols = lambda W: np.concatenate(
            [W[:, h * HD : (h + 1) * HD] for h in hs], axis=1
        )
        colsb = lambda b: np.concatenate(
            [b[h * HD : (h + 1) * HD] for h in hs], axis=0
        )
        wqkv_l = np.ascontiguousarray(
            np.concatenate([cols(Wq), cols(Wk), cols(Wv)], axis=1).astype(BF16)
        )
        bq_l = np.ascontiguousarray(colsb(bq).astype(np.float32))
        wo_l = np.ascontiguousarray(
            Wo[hs[0] * HD : hs[0] * HD + HPC * HD, :].astype(BF16)
        )
        in_maps.append(
            {
                "xt": xt,
                "wqkv": wqkv_l,
                "bq": bq_l,
                "wo": wo_l,
                "masks": masks,
            }
        )
    return in_maps


def kernel(input, Wqkv, bqkv, Wo, bo):
    from concourse.bass_utils import run_bass_kernel_spmd

    S = np.asarray(input).reshape(-1, D).shape[0]
    nc = _get_nc(S)
    in_maps = make_in_maps(input, Wqkv, bqkv, Wo, S)
    res = None
    last_exc = None
    for _attempt in range(3):  # transient NRT/device errors: retry
        try:
            res = run_bass_kernel_spmd(nc, in_maps, core_ids=list(range(NCORES)))
            break
        except Exception as e:  # noqa: BLE001
            last_exc = e
    if res is None:
        raise last_exc
    yt = res.results[0]["yt"].astype(np.float32)
    for r in res.results[1:]:
        yt += r["yt"].astype(np.float32)
    # fold the V bias through the output projection: y += bv @ Wo + bo
    bv = np.asarray(bqkv, dtype=np.float32)[2 * D : 3 * D]
    bo_eff = np.asarray(bo, dtype=np.float32) + bv @ np.asarray(
        Wo, dtype=np.float32
    )
    y = yt.T + bo_eff[None, :]
    return np.ascontiguousarray(y, dtype=np.float32).reshape(1, S, D)
